# revision 34
# baseline (speedup 1.0000x reference)
"""Trainium2 Bass kernel for nn_FCLModule_74131135529089 (moe_routing).

Module structure (B=262144 rows, input dim 1):
    circle/rect expert towers 1->32->64->256 (relu, zero biases)
    per-row select by shape_type, stage2 256->256 relu + residual,
    stage3 256->512 relu, 512->512, LayerNorm(512).

All bias vectors in this module are zero and every stage before the
LayerNorm is therefore positively homogeneous in x: for each row,
    h2(x) = |x| * H[k],   k = 2*shape_type + (x < 0),
where H[k] in R^512 is the full pre-LayerNorm output of the network
evaluated at x = +-1 for each expert.  The LayerNorm then collapses to
    out = C[k] * t + ln_b,  C[k] = (H[k]-mean(H[k]))*ln_g,
    t = |x| / sqrt(x^2 * var(H[k]) + eps).
The device kernel computes t/masks per row, forms a [rows,K] one-hot*t
matrix per 128-row chunk (fp16 hi/lo split rows, K padded to 32) and
multiplies it with the constant [K,512] matrix (C rows and ln_b) on the
tensor engine, streaming the output as fp16 (256 MB) at the HBM write
roofline; the host upcasts to f32 during the gather.  If any
structural assumption is violated (nonzero biases / shape_type outside
{0,1}) we fall back to a dense numpy evaluation of the module.

Sharding: pure data parallel over the batch dim, 8 cores x 32768 rows.
"""

import numpy as np

B = 262144
TD = 512
N_CORES = 8
RPC = B // N_CORES          # rows per core = 32768
P = 128                     # SBUF partitions
CPB = RPC // P              # columns per partition = 256 (row r = p*CPB + j)
G = 16                      # 128-row chunks per output DMA (2 MB per DMA)
EPS = 1e-5
# Matmul operands are fp16 with an error-compensating split:
#   t*C = t_hi@C_hi + t_hi@C_lo + t_lo@C_hi   (+ ln_b hi/lo)
# folded into one K=14 matmul; fp16 streams at full PE rate (fp32 is 4x
# slower, float32r is ~11-bit) and the split recovers ~22 mantissa bits.
# K is padded to 16 so EIGHT 128-row chunks batch into one [128,128]
# PE transpose (half the transposes and PSUM->SBUF lhs copies of a
# K=32 layout).  Matmul start partitions must be 32-aligned, so chunk
# PAIRS run as K=32 row-tiled matmuls against two zero-banded constant
# matrices (cmA = [cm;0] bands hits the even chunk, cmB = [0;cm] the
# odd one); the extra MACs land on zeros and PE has headroom.
KDIM = 16
TB = 8                      # chunks per transpose batch
# PSUM->SBUF drain split: ACT_NUM of every SPLIT_DEN chunk-pair copies
# go to the scalar engine, the rest to the vector engine (both drain
# ~1.1us per [P,2,TD] pair; DVE also carries the lhs copies).
ACT_NUM = 72
SPLIT_DEN = 128

_CACHE: dict = {}


def _towers_collapse(inputs):
    """Host-side constant folding (float64): returns the replicated fp16
    constant matrix [128,TD] and sig2 [4] f64, for k = 2*shape_type + (x<0)
    in order (c,+),(c,-),(r,+),(r,-)."""
    W = {k: np.asarray(v, dtype=np.float64) for k, v in inputs.items()}
    H = []
    for e in ("c", "r"):
        for sign in (1.0, -1.0):
            v = np.array([[sign]])
            for li in ("1", "2", "3"):
                v = np.maximum(v @ W[e + "w" + li] + W[e + "b" + li], 0.0)
            x2 = np.maximum(v @ W["s2w"] + W["s2b"], 0.0) + v
            h = np.maximum(x2 @ W["w3a"] + W["b3a"], 0.0)
            H.append((h @ W["w3b"] + W["b3b"])[0])
    # reorder to k = 2*s + neg: (c,+),(c,-),(r,+),(r,-) == H[0],H[1],H[2],H[3]
    H = np.stack(H)                                   # [4, TD]
    mu = H.mean(axis=1, keepdims=True)
    sig2 = H.var(axis=1)                              # [4]
    C = (H - mu) * W["ln_g"][None, :]                 # [4, TD]
    lnb = W["ln_b"]
    # Difference basis: value(n,s) = D0 + n*D1 + s*D2 + n*s*D3 with
    # n = (x<0), s = shape_type.  The device lhs planes are then just
    # {t, t*n, t*s, t*n*s} -- pure multiplies/copies, which is the full
    # extent of what the GpSimd engine supports (no add/sub on Pool).
    D = np.zeros_like(C)
    D[0] = C[0]
    D[1] = C[1] - C[0]
    D[2] = C[2] - C[0]
    D[3] = C[3] - C[1] - C[2] + C[0]
    D_hi = D.astype(np.float16)
    D_lo = (D - D_hi.astype(np.float64)).astype(np.float16)
    b_hi = lnb.astype(np.float16)
    b_lo = (lnb - b_hi.astype(np.float64)).astype(np.float16)
    # rows: 0-3 D_hi (x t_hi planes), 4-7 D_lo (x t_hi planes),
    #       8-11 D_hi (x t_lo planes), 12 b_hi, 13 b_lo, 14-15 zero pad
    cmat = np.zeros((KDIM, TD), np.float16)
    cmat[0:4] = D_hi
    cmat[4:8] = D_lo
    cmat[8:12] = D_hi
    cmat[12] = b_hi
    cmat[13] = b_lo
    # Zero-banded variants for 32-aligned chunk-pair matmuls: band i of
    # cmA holds [cmat; 0] (even chunk of the pair), cmB holds [0; cmat].
    z = np.zeros((KDIM, TD), np.float16)
    cmA = np.tile(np.vstack([cmat, z]), (4, 1))       # [128, TD] f16
    cmB = np.tile(np.vstack([z, cmat]), (4, 1))       # [128, TD] f16
    return np.ascontiguousarray(cmA), np.ascontiguousarray(cmB), sig2


def _assumptions_hold(inputs):
    for name in ("cb1", "cb2", "cb3", "rb1", "rb2", "rb3", "s2b", "b3a", "b3b"):
        if np.any(np.asarray(inputs[name]) != 0):
            return False
    st = np.asarray(inputs["shape_type"])
    if not np.isin(st, (0, 1)).all():
        return False
    x = np.asarray(inputs["x"])
    return bool(np.isfinite(x).all()) and x.shape == (B, 1) and st.shape == (B, 1)


def _fallback_numpy(inputs):
    f = {k: np.asarray(v, dtype=np.float32) for k, v in inputs.items()}

    def tower(h, w1, b1, w2, b2, w3, b3):
        h = np.maximum(h @ w1 + b1, 0)
        h = np.maximum(h @ w2 + b2, 0)
        return np.maximum(h @ w3 + b3, 0)

    x = f["x"]
    circle = tower(x, f["cw1"], f["cb1"], f["cw2"], f["cb2"], f["cw3"], f["cb3"])
    rect = tower(x, f["rw1"], f["rb1"], f["rw2"], f["rb2"], f["rw3"], f["rb3"])
    mask = np.asarray(inputs["shape_type"]) < 0.5
    x1 = np.where(mask, circle, rect)
    x2 = np.maximum(x1 @ f["s2w"] + f["s2b"], 0) + x1
    h = np.maximum(x2 @ f["w3a"] + f["b3a"], 0)
    h = h @ f["w3b"] + f["b3b"]
    mu = h.mean(axis=-1, keepdims=True)
    var = h.var(axis=-1, keepdims=True)
    return ((h - mu) / np.sqrt(var + EPS) * f["ln_g"] + f["ln_b"]).astype(np.float32)


def _build_nc(sig2, reps=1):
    import concourse.bacc as bacc
    import concourse.bass as bass
    import concourse.mybir as mybir
    import concourse.tile as tile

    f32 = mybir.dt.float32
    f16 = mybir.dt.float16
    a = float(sig2[0])
    b = float(sig2[1] - sig2[0])
    c = float(sig2[2] - sig2[0])
    d = float(sig2[3] - sig2[2] - sig2[1] + sig2[0])
    mul = mybir.AluOpType.mult
    add = mybir.AluOpType.add
    sub = mybir.AluOpType.subtract

    nc = bacc.Bacc("TRN2", target_bir_lowering=False, debug=False,
                   num_devices=N_CORES)
    x_d = nc.dram_tensor("x", [P, CPB], f32, kind="ExternalInput").ap()
    s_d = nc.dram_tensor("st", [P, CPB], f32, kind="ExternalInput").ap()
    a_d = nc.dram_tensor("cmA", [P, TD], f16, kind="ExternalInput").ap()
    b_d = nc.dram_tensor("cmB", [P, TD], f16, kind="ExternalInput").ap()
    i_d = nc.dram_tensor("ident", [P, P], f16, kind="ExternalInput").ap()
    # fp16 output stream: halves HBM write traffic (the roofline); host
    # upcasts to f32 after gather.  Final-store rounding is ~2^-11
    # relative, far inside the accuracy gate.
    y_d = nc.dram_tensor("y", [P, CPB, TD], f16, kind="ExternalOutput").ap()

    with tile.TileContext(nc) as tc:
        with (
            tc.tile_pool(name="const", bufs=1) as const,
            tc.tile_pool(name="pre", bufs=1) as pre,
            tc.tile_pool(name="lhs", bufs=6) as lhsp,
            tc.tile_pool(name="outs", bufs=3) as outp,
            tc.tile_pool(name="ps_t", bufs=2, space="PSUM") as ps_t,
            tc.tile_pool(name="ps_o", bufs=2, space="PSUM") as ps_o,
        ):
            xr = pre.tile([P, CPB], f32)
            sf = pre.tile([P, CPB], f32)
            neg = pre.tile([P, CPB], f32)
            sn = pre.tile([P, CPB], f32)
            u1 = pre.tile([P, CPB], f32)
            u2 = pre.tile([P, CPB], f32)
            sg = pre.tile([P, CPB], f32)
            x2 = pre.tile([P, CPB], f32)
            ve = pre.tile([P, CPB], f32)
            ve2 = pre.tile([P, CPB], f32)
            rc = pre.tile([P, CPB], f32)
            t2 = pre.tile([P, CPB], f32)
            tt = pre.tile([P, CPB], f32)
            m4 = pre.tile([P, CPB, KDIM], f16)

            # The PSUM f32 -> SBUF f16 drain (16.7M elem/core) is the
            # engine-side critical path under the fp16 DMA stream: both
            # ACT and DVE drain ~1 elem/cycle/lane, so chunk-pair copies
            # are interleaved ACT_NUM:SPLIT_DEN-ACT_NUM between them
            # (DVE also carries the lhs copies; everything else is off
            # these two engines).
            # [total chunks drained, chunks drained by ACT]; seeded
            # so the first drains land on ACT, which is idle during the
            # ramp while DVE clears prep work
            drain_state = [0, -4]

            def drain(dst, pp, nch):
                # chunk-weighted split between ACT and DVE: send this
                # set to whichever engine is furthest below its target
                # share of drained chunks
                total, act = drain_state
                if (act + nch) * SPLIT_DEN <= (total + nch) * ACT_NUM:
                    drain_state[1] = act + nch
                    nc.scalar.copy(dst, pp)
                else:
                    nc.vector.tensor_copy(dst, pp)
                drain_state[0] = total + nch

            def mm(pp_col, lh, pos, cmx):
                band = 32 * (pos // 2)
                nc.tensor.matmul(
                    pp_col, lh[band:band + 32, :], cmx[band:band + 32, :],
                    start=True, stop=True, tile_position=(band, 0))

            def emit_group(g0, gsz):
                outt = outp.tile([P, G, TD], f16, tag="outt")
                lhs = []
                for j8 in range(0, gsz, TB):
                    nb = min(TB, gsz - j8)
                    tp = ps_t.tile([P, P], f16, tag="tp")
                    nc.tensor.transpose(
                        tp[:KDIM * nb], m4[:, g0 + j8:g0 + j8 + nb, :],
                        idt[:])
                    lh = lhsp.tile([P, P], f16, tag="lh")
                    nc.vector.tensor_copy(lh[:KDIM * nb], tp[:KDIM * nb])
                    lhs.append(lh)
                if gsz == G:
                    # triple/pair drains over cross-band chunk sets: the
                    # matmul sequence rotates the four PE row bands
                    # (0,32,64 / 0,32,64 / 96,0,32 / 96,0,32 / 64,96 /
                    # 64,96) so consecutive matmuls stream concurrently,
                    # while each PSUM tile drains 3 (or 2) chunks in one
                    # ACT/DVE copy.  cmA/cmB zero banding selects the
                    # chunk's 16 planes within its shared 32-row band.
                    sets = ((0, 2, 4), (1, 3, 5), (6, 8, 10), (7, 9, 11),
                            (12, 14), (13, 15))
                else:
                    sets = tuple((a, a + 1) for a in range(0, gsz, 2))
                for cs_ in sets:
                    nch = len(cs_)
                    pp = ps_o.tile([P, 3, TD], f32, tag="po")
                    for k, c in enumerate(cs_):
                        pos = c % TB
                        lh = lhs[c // TB]
                        mm(pp[:, k, :], lh, pos,
                           cmA if pos % 2 == 0 else cmB)
                    dl = cs_[1] - cs_[0]
                    drain(outt[:, cs_[0]:cs_[-1] + 1:dl, :],
                          pp[:, 0:nch, :], nch)
                nc.sync.dma_start(y_d[:, g0:g0 + gsz, :], outt[:, 0:gsz, :])

            nc.sync.dma_start(xr[:], x_d[:])
            nc.scalar.dma_start(sf[:], s_d[:])
            cmA = const.tile([P, TD], f16)
            nc.scalar.dma_start(cmA[:], a_d[:])
            cmB = const.tile([P, TD], f16)
            nc.scalar.dma_start(cmB[:], b_d[:])
            idt = const.tile([P, P], f16)
            nc.sync.dma_start(idt[:], i_d[:])

            # constant planes, valid for every column: 8-11 zero
            # (dropped t_lo planes), 12/13 all-ones (bias rows), 14/15
            # zero pad; cheap one-time Pool memsets
            nc.gpsimd.memset(m4[:, :, 8:12], 0.0)
            nc.gpsimd.memset(m4[:, :, 12:14], 1.0)
            nc.gpsimd.memset(m4[:, :, 14:KDIM], 0.0)

            # Per-row preprocessing.  The Pool (GpSimd) engine only
            # supports mult/copy, so it carries exactly those (the m4
            # plane products are the bulk); DVE keeps the few
            # add/sub/compare ops, ACT the +eps and Sqrt.  Slice 0 runs
            # fully on DVE for a fast ramp.  Each later slice is split
            # into three emission phases (pre / mid / post) placed
            # between groups so the short DVE/ACT mid-chain ops sit in
            # their FIFOs close to where their cross-engine inputs
            # complete, without blocking queued drains for long.
            bounds = [0, 32, 96, 176, CPB]
            nslices = len(bounds) - 1

            def prep_pre(h):
                e = nc.vector if h == 0 else nc.gpsimd
                cs = slice(bounds[h], bounds[h + 1])
                nc.vector.tensor_scalar(neg[:, cs], xr[:, cs], 0.0, None,
                                        mybir.AluOpType.is_lt)
                # sig2 per row: a + b*neg + c*sf + d*sn
                nc.vector.tensor_scalar(u1[:, cs], sf[:, cs], c, a, mul, add)
                nc.vector.scalar_tensor_tensor(u2[:, cs], neg[:, cs], b,
                                               u1[:, cs], mul, add)
                e.tensor_tensor(sn[:, cs], sf[:, cs], neg[:, cs], mul)
                nc.vector.scalar_tensor_tensor(sg[:, cs], sn[:, cs], d,
                                               u2[:, cs], mul, add)
                e.tensor_tensor(x2[:, cs], xr[:, cs], xr[:, cs], mul)
                e.tensor_tensor(ve[:, cs], x2[:, cs], sg[:, cs], mul)

            def prep_mid(h):
                e = nc.vector if h == 0 else nc.gpsimd
                cs = slice(bounds[h], bounds[h + 1])
                # t = sqrt(x^2 / (x^2*sig2 + eps))
                nc.scalar.activation(ve2[:, cs], ve[:, cs],
                                     mybir.ActivationFunctionType.Copy,
                                     bias=EPS, scale=1.0)
                nc.vector.reciprocal(rc[:, cs], ve2[:, cs])
                e.tensor_tensor(t2[:, cs], x2[:, cs], rc[:, cs], mul)
                nc.scalar.activation(tt[:, cs], t2[:, cs],
                                     mybir.ActivationFunctionType.Sqrt)

            def prep_post(h):
                e = nc.vector if h == 0 else nc.gpsimd
                cs = slice(bounds[h], bounds[h + 1])
                # planes (difference basis): {t, t*n, t*s, t*n*s} for
                # D_hi, duplicated for D_lo.  (No t_lo compensation:
                # t_hi rounding costs <= 2^-11 relative, inside the
                # accuracy budget, and dropping it removes the whole
                # th16/th32/tl cross-engine chain.)
                e.tensor_copy(m4[:, cs, 0], tt[:, cs])
                e.tensor_tensor(m4[:, cs, 1], tt[:, cs], neg[:, cs], mul)
                e.tensor_tensor(m4[:, cs, 2], tt[:, cs], sf[:, cs], mul)
                e.tensor_tensor(m4[:, cs, 3], tt[:, cs], sn[:, cs], mul)
                e.tensor_copy(m4[:, cs, 4:8], m4[:, cs, 0:4])

            def slice_groups(h):
                # ramped small at the start (first DMA out early),
                # small again at the very end (short drain+DMA tail)
                if h == 0:
                    groups = [(0, 2), (2, 2), (4, 4), (8, 8)]
                    g = 16
                else:
                    groups = []
                    g = bounds[h]
                end = bounds[h + 1] if h + 1 < nslices else CPB - 16
                while g < end:
                    groups.append((g, G))
                    g += G
                if h == nslices - 1:
                    groups += [(g, 8), (g + 8, 4), (g + 12, 2),
                               (g + 14, 2)]
                return groups

            prep_pre(0)
            prep_mid(0)
            prep_post(0)
            for h in range(nslices):
                gs = slice_groups(h)
                for i, (g0, gsz) in enumerate(gs):
                    emit_group(g0, gsz)
                    if h + 1 < nslices:
                        # stage the next slice's prep across this
                        # slice's first few groups; for late slices,
                        # wait-gate the scheduler (simulated-time lower
                        # bound, no hardware cost) so their mid-chain
                        # ACT/DVE ops can't be hoisted ahead of early
                        # drains and head-of-line block those engines
                        gate = max(0.0, bounds[h + 1] * 0.35e-3
                                   - 14e-3)
                        if i == 0:
                            prep_pre(h + 1)
                        elif i == 1:
                            with tc.tile_wait_until(gate, enable=gate > 0):
                                prep_mid(h + 1)
                        elif i == 2:
                            with tc.tile_wait_until(gate, enable=gate > 0):
                                prep_post(h + 1)

            # extra full passes for repeat-based HW timing (reps > 1)
            for _ in range(reps - 1):
                for g0 in range(0, CPB, G):
                    emit_group(g0, G)
    nc.compile()
    return nc


def _make_in_maps(inputs, cmA, cmB):
    x = np.ascontiguousarray(np.asarray(inputs["x"], dtype=np.float32)).reshape(B)
    st = np.asarray(inputs["shape_type"]).astype(np.float32).reshape(B)
    ident = np.eye(P, dtype=np.float16)
    in_maps = []
    for i in range(N_CORES):
        sl = slice(i * RPC, (i + 1) * RPC)
        in_maps.append({
            "x": x[sl].reshape(P, CPB).copy(),
            "st": st[sl].reshape(P, CPB).copy(),
            "cmA": cmA,
            "cmB": cmB,
            "ident": ident,
        })
    return in_maps


def _get_nc(sig2):
    key = tuple(np.round(sig2, 12))
    if key not in _CACHE:
        _CACHE[key] = _build_nc(sig2)
    return _CACHE[key]


def _get_runner(nc):
    """Cached jit-compiled SPMD executor for `nc` (same mechanics as
    concourse.bass2jax.run_bass_via_pjrt, memoized so repeated kernel()
    calls skip jax re-tracing)."""
    if hasattr(nc, "_cached_runner"):
        return nc._cached_runner
    import jax
    from jax.experimental.shard_map import shard_map
    from jax.sharding import Mesh, PartitionSpec

    import concourse.mybir as mybir
    from concourse import bass2jax

    bass2jax.install_neuronx_cc_hook()

    part_name = (nc.partition_id_tensor.name
                 if nc.partition_id_tensor else None)
    in_names, out_names, out_avals = [], [], []
    for alloc in nc.m.functions[0].allocations:
        if not isinstance(alloc, mybir.MemoryLocationSet):
            continue
        name = alloc.memorylocations[0].name
        if alloc.kind == "ExternalInput":
            if name != part_name:
                in_names.append(name)
        elif alloc.kind == "ExternalOutput":
            out_names.append(name)
            out_avals.append(jax.core.ShapedArray(
                tuple(alloc.tensor_shape), mybir.dt.np(alloc.dtype)))
    n_params = len(in_names)
    all_names = in_names + out_names
    if part_name is not None:
        all_names = all_names + [part_name]
    donate = tuple(range(n_params, n_params + len(out_names)))

    def _body(*args):
        operands = list(args)
        if part_name is not None:
            operands.append(bass2jax.partition_id_tensor())
        return tuple(bass2jax._bass_exec_p.bind(
            *operands,
            out_avals=tuple(out_avals),
            in_names=tuple(all_names),
            out_names=tuple(out_names),
            lowering_input_output_aliases=(),
            sim_require_finite=True,
            sim_require_nnan=True,
            nc=nc,
        ))

    devices = jax.devices()[:N_CORES]
    mesh = Mesh(np.asarray(devices), ("core",))
    sharded = jax.jit(
        shard_map(_body, mesh=mesh,
                  in_specs=(PartitionSpec("core"),) * (n_params + len(out_names)),
                  out_specs=(PartitionSpec("core"),) * len(out_names),
                  check_rep=False),
        donate_argnums=donate, keep_unused=True)
    runner = (sharded, in_names, out_names, out_avals)
    nc._cached_runner = runner
    return runner


def _run_spmd(nc, in_maps):
    sharded, in_names, out_names, out_avals = _get_runner(nc)
    concat_in = [
        np.concatenate([np.asarray(m[name])[None] for m in in_maps], axis=0)
        .reshape(N_CORES * in_maps[0][name].shape[0],
                 *in_maps[0][name].shape[1:])
        for name in in_names
    ]
    concat_zeros = [
        np.zeros((N_CORES * a.shape[0], *a.shape[1:]), a.dtype)
        for a in out_avals
    ]
    out_arrs = sharded(*concat_in, *concat_zeros)
    return {
        name: np.asarray(out_arrs[i]).reshape(
            N_CORES, *out_avals[i].shape)
        for i, name in enumerate(out_names)
    }


def kernel(**inputs) -> np.ndarray:
    if not _assumptions_hold(inputs):
        return _fallback_numpy(inputs)

    cmA, cmB, sig2 = _towers_collapse(inputs)
    nc = _get_nc(sig2)
    in_maps = _make_in_maps(inputs, cmA, cmB)
    y = _run_spmd(nc, in_maps)["y"]            # [N_CORES, P, CPB, TD] f16
    return y.reshape(B, TD).astype(np.float32)



# revision 35
# speedup vs baseline: 1.2751x; 1.2751x over previous
"""Trainium2 Bass kernel for nn_FCLModule_74131135529089 (moe_routing).

Module structure (B=262144 rows, input dim 1):
    circle/rect expert towers 1->32->64->256 (relu, zero biases)
    per-row select by shape_type, stage2 256->256 relu + residual,
    stage3 256->512 relu, 512->512, LayerNorm(512).

All bias vectors in this module are zero and every stage before the
LayerNorm is therefore positively homogeneous in x: for each row,
    h2(x) = |x| * H[k],   k = 2*shape_type + (x < 0),
where H[k] in R^512 is the full pre-LayerNorm output of the network
evaluated at x = +-1 for each expert.  The LayerNorm then collapses to
    out = C[k] * t + ln_b,  C[k] = (H[k]-mean(H[k]))*ln_g,
    t = |x| / sqrt(x^2 * var(H[k]) + eps).
The device kernel computes t/masks per row, forms a [rows,K] one-hot*t
matrix per 128-row chunk (fp16 hi/lo split rows, K padded to 32) and
multiplies it with the constant [K,512] matrix (C rows and ln_b) on the
tensor engine, streaming the output as fp16 (256 MB) at the HBM write
roofline; the host upcasts to f32 during the gather.  If any
structural assumption is violated (nonzero biases / shape_type outside
{0,1}) we fall back to a dense numpy evaluation of the module.

Sharding: pure data parallel over the batch dim, 8 cores x 32768 rows.
"""

import numpy as np

B = 262144
TD = 512
N_CORES = 8
RPC = B // N_CORES          # rows per core = 32768
P = 128                     # SBUF partitions
CPB = RPC // P              # columns per partition = 256 (row r = p*CPB + j)
G = 8                       # 128-row chunks per output DMA (1 MB per DMA)
EPS = 1e-5
# Matmul operands are fp16 with an error-compensating split:
#   t*C = t_hi@C_hi + t_hi@C_lo + t_lo@C_hi   (+ ln_b hi/lo)
# folded into one K=14 matmul; fp16 streams at full PE rate (fp32 is 4x
# slower, float32r is ~11-bit) and the split recovers ~22 mantissa bits.
# K is padded to 16 so EIGHT 128-row chunks batch into one [128,128]
# PE transpose (half the transposes and PSUM->SBUF lhs copies of a
# K=32 layout).  Matmul start partitions must be 32-aligned, so chunk
# PAIRS run as K=32 row-tiled matmuls against two zero-banded constant
# matrices (cmA = [cm;0] bands hits the even chunk, cmB = [0;cm] the
# odd one); the extra MACs land on zeros and PE has headroom.
KDIM = 16
TB = 8                      # chunks per transpose batch
# PSUM->SBUF drain split: ACT_NUM of every SPLIT_DEN chunk-pair copies
# go to the scalar engine, the rest to the vector engine (both drain
# ~1.1us per [P,2,TD] pair; DVE also carries the lhs copies).
ACT_NUM = 72
SPLIT_DEN = 128

_CACHE: dict = {}


def _towers_collapse(inputs):
    """Host-side constant folding (float64): returns the replicated fp16
    constant matrix [128,TD] and sig2 [4] f64, for k = 2*shape_type + (x<0)
    in order (c,+),(c,-),(r,+),(r,-)."""
    W = {k: np.asarray(v, dtype=np.float64) for k, v in inputs.items()}
    H = []
    for e in ("c", "r"):
        for sign in (1.0, -1.0):
            v = np.array([[sign]])
            for li in ("1", "2", "3"):
                v = np.maximum(v @ W[e + "w" + li] + W[e + "b" + li], 0.0)
            x2 = np.maximum(v @ W["s2w"] + W["s2b"], 0.0) + v
            h = np.maximum(x2 @ W["w3a"] + W["b3a"], 0.0)
            H.append((h @ W["w3b"] + W["b3b"])[0])
    # reorder to k = 2*s + neg: (c,+),(c,-),(r,+),(r,-) == H[0],H[1],H[2],H[3]
    H = np.stack(H)                                   # [4, TD]
    mu = H.mean(axis=1, keepdims=True)
    sig2 = H.var(axis=1)                              # [4]
    C = (H - mu) * W["ln_g"][None, :]                 # [4, TD]
    lnb = W["ln_b"]
    # Difference basis: value(n,s) = D0 + n*D1 + s*D2 + n*s*D3 with
    # n = (x<0), s = shape_type.  The device lhs planes are then just
    # {t, t*n, t*s, t*n*s} -- pure multiplies/copies, which is the full
    # extent of what the GpSimd engine supports (no add/sub on Pool).
    D = np.zeros_like(C)
    D[0] = C[0]
    D[1] = C[1] - C[0]
    D[2] = C[2] - C[0]
    D[3] = C[3] - C[1] - C[2] + C[0]
    D_hi = D.astype(np.float16)
    D_lo = (D - D_hi.astype(np.float64)).astype(np.float16)
    b_hi = lnb.astype(np.float16)
    b_lo = (lnb - b_hi.astype(np.float64)).astype(np.float16)
    # rows: 0-3 D_hi (x t_hi planes), 4-7 D_lo (x t_hi planes),
    #       8-11 D_hi (x t_lo planes), 12 b_hi, 13 b_lo, 14-15 zero pad
    cmat = np.zeros((KDIM, TD), np.float16)
    cmat[0:4] = D_hi
    cmat[4:8] = D_lo
    cmat[8:12] = D_hi
    cmat[12] = b_hi
    cmat[13] = b_lo
    # Zero-banded variants for 32-aligned chunk-pair matmuls: band i of
    # cmA holds [cmat; 0] (even chunk of the pair), cmB holds [0; cmat].
    z = np.zeros((KDIM, TD), np.float16)
    cmA = np.tile(np.vstack([cmat, z]), (4, 1))       # [128, TD] f16
    cmB = np.tile(np.vstack([z, cmat]), (4, 1))       # [128, TD] f16
    return np.ascontiguousarray(cmA), np.ascontiguousarray(cmB), sig2


def _assumptions_hold(inputs):
    for name in ("cb1", "cb2", "cb3", "rb1", "rb2", "rb3", "s2b", "b3a", "b3b"):
        if np.any(np.asarray(inputs[name]) != 0):
            return False
    st = np.asarray(inputs["shape_type"])
    if not np.isin(st, (0, 1)).all():
        return False
    x = np.asarray(inputs["x"])
    return bool(np.isfinite(x).all()) and x.shape == (B, 1) and st.shape == (B, 1)


def _fallback_numpy(inputs):
    f = {k: np.asarray(v, dtype=np.float32) for k, v in inputs.items()}

    def tower(h, w1, b1, w2, b2, w3, b3):
        h = np.maximum(h @ w1 + b1, 0)
        h = np.maximum(h @ w2 + b2, 0)
        return np.maximum(h @ w3 + b3, 0)

    x = f["x"]
    circle = tower(x, f["cw1"], f["cb1"], f["cw2"], f["cb2"], f["cw3"], f["cb3"])
    rect = tower(x, f["rw1"], f["rb1"], f["rw2"], f["rb2"], f["rw3"], f["rb3"])
    mask = np.asarray(inputs["shape_type"]) < 0.5
    x1 = np.where(mask, circle, rect)
    x2 = np.maximum(x1 @ f["s2w"] + f["s2b"], 0) + x1
    h = np.maximum(x2 @ f["w3a"] + f["b3a"], 0)
    h = h @ f["w3b"] + f["b3b"]
    mu = h.mean(axis=-1, keepdims=True)
    var = h.var(axis=-1, keepdims=True)
    return ((h - mu) / np.sqrt(var + EPS) * f["ln_g"] + f["ln_b"]).astype(np.float32)


def _build_nc(sig2, reps=1):
    import concourse.bacc as bacc
    import concourse.bass as bass
    import concourse.mybir as mybir
    import concourse.tile as tile

    f32 = mybir.dt.float32
    f16 = mybir.dt.float16
    a = float(sig2[0])
    b = float(sig2[1] - sig2[0])
    c = float(sig2[2] - sig2[0])
    d = float(sig2[3] - sig2[2] - sig2[1] + sig2[0])
    mul = mybir.AluOpType.mult
    add = mybir.AluOpType.add
    sub = mybir.AluOpType.subtract

    nc = bacc.Bacc("TRN2", target_bir_lowering=False, debug=False,
                   num_devices=N_CORES)
    x_d = nc.dram_tensor("x", [P, CPB], f32, kind="ExternalInput").ap()
    s_d = nc.dram_tensor("st", [P, CPB], f32, kind="ExternalInput").ap()
    a_d = nc.dram_tensor("cmA", [P, TD], f16, kind="ExternalInput").ap()
    b_d = nc.dram_tensor("cmB", [P, TD], f16, kind="ExternalInput").ap()
    i_d = nc.dram_tensor("ident", [P, P], f16, kind="ExternalInput").ap()
    # fp16 output stream: halves HBM write traffic (the roofline); host
    # upcasts to f32 after gather.  Final-store rounding is ~2^-11
    # relative, far inside the accuracy gate.
    y_d = nc.dram_tensor("y", [P, CPB, TD], f16, kind="ExternalOutput").ap()

    with tile.TileContext(nc) as tc:
        with (
            tc.tile_pool(name="const", bufs=1) as const,
            tc.tile_pool(name="pre", bufs=1) as pre,
            tc.tile_pool(name="lhs", bufs=6) as lhsp,
            tc.tile_pool(name="outs", bufs=3) as outp,
            tc.tile_pool(name="ps_t", bufs=2, space="PSUM") as ps_t,
            tc.tile_pool(name="ps_o", bufs=3, space="PSUM") as ps_o,
        ):
            xr = pre.tile([P, CPB], f32)
            sf = pre.tile([P, CPB], f32)
            neg = pre.tile([P, CPB], f32)
            sn = pre.tile([P, CPB], f32)
            u1 = pre.tile([P, CPB], f32)
            u2 = pre.tile([P, CPB], f32)
            sg = pre.tile([P, CPB], f32)
            x2 = pre.tile([P, CPB], f32)
            ve = pre.tile([P, CPB], f32)
            ve2 = pre.tile([P, CPB], f32)
            rc = pre.tile([P, CPB], f32)
            t2 = pre.tile([P, CPB], f32)
            tt = pre.tile([P, CPB], f32)
            m4 = pre.tile([P, CPB, KDIM], f16)

            # The PSUM f32 -> SBUF f16 drain (16.7M elem/core) is the
            # engine-side critical path under the fp16 DMA stream: both
            # ACT and DVE drain ~1 elem/cycle/lane, so chunk-pair copies
            # are interleaved ACT_NUM:SPLIT_DEN-ACT_NUM between them
            # (DVE also carries the lhs copies; everything else is off
            # these two engines).
            # [total chunks drained, chunks drained by ACT]; seeded
            # so the first drains land on ACT, which is idle during the
            # ramp while DVE clears prep work
            drain_state = [0, -4]

            def drain(dst, pp, nch):
                # chunk-weighted split between ACT and DVE: send this
                # set to whichever engine is furthest below its target
                # share of drained chunks
                total, act = drain_state
                if (act + nch) * SPLIT_DEN <= (total + nch) * ACT_NUM:
                    drain_state[1] = act + nch
                    nc.scalar.copy(dst, pp)
                else:
                    nc.vector.tensor_copy(dst, pp)
                drain_state[0] = total + nch

            def mm(pp_col, lh, pos, cmx):
                band = 32 * (pos // 2)
                nc.tensor.matmul(
                    pp_col, lh[band:band + 32, :], cmx[band:band + 32, :],
                    start=True, stop=True, tile_position=(band, 0))

            def emit_group(g0, gsz):
                outt = outp.tile([P, G, TD], f16, tag="outt")
                tp = ps_t.tile([P, P], f16, tag="tp")
                nc.tensor.transpose(
                    tp[:KDIM * gsz], m4[:, g0:g0 + gsz, :], idt[:])
                lh = lhsp.tile([P, P], f16, tag="lh")
                nc.vector.tensor_copy(lh[:KDIM * gsz], tp[:KDIM * gsz])
                if gsz == G:
                    # Cross-band pairing (0,2),(4,6),(1,3),(5,7): each
                    # PSUM pair spans two different PE row bands and the
                    # matmul sequence rotates 0,32,64,96,... so
                    # consecutive matmuls stream concurrently instead of
                    # serializing on one band.  cmA/cmB zero banding
                    # selects the chunk's 16 planes within its band.
                    pairs = ((0, 2), (4, 6), (1, 3), (5, 7))
                else:
                    pairs = tuple((a, a + 1) for a in range(0, gsz, 2))
                for ca, cb in pairs:
                    pp = ps_o.tile([P, 2, TD], f32, tag="po")
                    mm(pp[:, 0, :], lh, ca, cmA if ca % 2 == 0 else cmB)
                    mm(pp[:, 1, :], lh, cb, cmA if cb % 2 == 0 else cmB)
                    dl = cb - ca
                    drain(outt[:, ca:cb + 1:dl, :], pp[:], 2)
                # alternate output DMAs between the Sync HWDGE queue and
                # the (otherwise idle) GpSimd SWDGE queue so per-DMA
                # issue latency pipelines across two issuing engines
                if (g0 // G) % 2 == 0:
                    nc.sync.dma_start(y_d[:, g0:g0 + gsz, :],
                                      outt[:, 0:gsz, :])
                else:
                    nc.gpsimd.dma_start(y_d[:, g0:g0 + gsz, :],
                                        outt[:, 0:gsz, :])

            nc.sync.dma_start(xr[:], x_d[:])
            nc.scalar.dma_start(sf[:], s_d[:])
            cmA = const.tile([P, TD], f16)
            nc.scalar.dma_start(cmA[:], a_d[:])
            cmB = const.tile([P, TD], f16)
            nc.scalar.dma_start(cmB[:], b_d[:])
            idt = const.tile([P, P], f16)
            nc.sync.dma_start(idt[:], i_d[:])

            # constant planes, valid for every column: 8-11 zero
            # (dropped t_lo planes), 12/13 all-ones (bias rows), 14/15
            # zero pad; cheap one-time Pool memsets
            nc.gpsimd.memset(m4[:, :, 8:12], 0.0)
            nc.gpsimd.memset(m4[:, :, 12:14], 1.0)
            nc.gpsimd.memset(m4[:, :, 14:KDIM], 0.0)

            # Per-row preprocessing.  The Pool (GpSimd) engine only
            # supports mult/copy, so it carries exactly those (the m4
            # plane products are the bulk); DVE keeps the few
            # add/sub/compare ops, ACT the +eps and Sqrt.  Slice 0 runs
            # fully on DVE for a fast ramp.  Each later slice is split
            # into three emission phases (pre / mid / post) placed
            # between groups so the short DVE/ACT mid-chain ops sit in
            # their FIFOs close to where their cross-engine inputs
            # complete, without blocking queued drains for long.
            bounds = [0, 32, 96, 176, CPB]
            nslices = len(bounds) - 1

            def prep_pre(h):
                e = nc.vector if h == 0 else nc.gpsimd
                cs = slice(bounds[h], bounds[h + 1])
                nc.vector.tensor_scalar(neg[:, cs], xr[:, cs], 0.0, None,
                                        mybir.AluOpType.is_lt)
                # sig2 per row: a + b*neg + c*sf + d*sn
                nc.vector.tensor_scalar(u1[:, cs], sf[:, cs], c, a, mul, add)
                nc.vector.scalar_tensor_tensor(u2[:, cs], neg[:, cs], b,
                                               u1[:, cs], mul, add)
                e.tensor_tensor(sn[:, cs], sf[:, cs], neg[:, cs], mul)
                nc.vector.scalar_tensor_tensor(sg[:, cs], sn[:, cs], d,
                                               u2[:, cs], mul, add)
                e.tensor_tensor(x2[:, cs], xr[:, cs], xr[:, cs], mul)
                e.tensor_tensor(ve[:, cs], x2[:, cs], sg[:, cs], mul)

            def prep_mid(h):
                e = nc.vector if h == 0 else nc.gpsimd
                cs = slice(bounds[h], bounds[h + 1])
                # t = sqrt(x^2 / (x^2*sig2 + eps))
                nc.scalar.activation(ve2[:, cs], ve[:, cs],
                                     mybir.ActivationFunctionType.Copy,
                                     bias=EPS, scale=1.0)
                nc.vector.reciprocal(rc[:, cs], ve2[:, cs])
                e.tensor_tensor(t2[:, cs], x2[:, cs], rc[:, cs], mul)
                nc.scalar.activation(tt[:, cs], t2[:, cs],
                                     mybir.ActivationFunctionType.Sqrt)

            def prep_post(h):
                e = nc.vector if h == 0 else nc.gpsimd
                cs = slice(bounds[h], bounds[h + 1])
                # planes (difference basis): {t, t*n, t*s, t*n*s} for
                # D_hi, duplicated for D_lo.  (No t_lo compensation:
                # t_hi rounding costs <= 2^-11 relative, inside the
                # accuracy budget, and dropping it removes the whole
                # th16/th32/tl cross-engine chain.)
                e.tensor_copy(m4[:, cs, 0], tt[:, cs])
                e.tensor_tensor(m4[:, cs, 1], tt[:, cs], neg[:, cs], mul)
                e.tensor_tensor(m4[:, cs, 2], tt[:, cs], sf[:, cs], mul)
                e.tensor_tensor(m4[:, cs, 3], tt[:, cs], sn[:, cs], mul)
                e.tensor_copy(m4[:, cs, 4:8], m4[:, cs, 0:4])

            def slice_groups(h):
                # ramped small at the start (first DMA out early),
                # small again at the very end (short drain+DMA tail)
                if h == 0:
                    groups = [(0, 2), (2, 2), (4, 4)]
                    g = 8
                else:
                    groups = []
                    g = bounds[h]
                end = bounds[h + 1] if h + 1 < nslices else CPB - 8
                while g < end:
                    groups.append((g, G))
                    g += G
                if h == nslices - 1:
                    groups += [(g, 4), (g + 4, 2), (g + 6, 2)]
                return groups

            prep_pre(0)
            prep_mid(0)
            prep_post(0)
            for h in range(nslices):
                gs = slice_groups(h)
                for i, (g0, gsz) in enumerate(gs):
                    emit_group(g0, gsz)
                    if h + 1 < nslices:
                        # stage the next slice's prep across this
                        # slice's first few groups; for late slices,
                        # wait-gate the scheduler (simulated-time lower
                        # bound, no hardware cost) so their mid-chain
                        # ACT/DVE ops can't be hoisted ahead of early
                        # drains and head-of-line block those engines
                        gate = max(0.0, bounds[h + 1] * 0.35e-3
                                   - 14e-3)
                        if i == 0:
                            prep_pre(h + 1)
                        elif i == 1:
                            with tc.tile_wait_until(gate, enable=gate > 0):
                                prep_mid(h + 1)
                        elif i == 2:
                            with tc.tile_wait_until(gate, enable=gate > 0):
                                prep_post(h + 1)

            # extra full passes for repeat-based HW timing (reps > 1)
            for _ in range(reps - 1):
                for g0 in range(0, CPB, G):
                    emit_group(g0, G)
    nc.compile()
    return nc


def _make_in_maps(inputs, cmA, cmB):
    x = np.ascontiguousarray(np.asarray(inputs["x"], dtype=np.float32)).reshape(B)
    st = np.asarray(inputs["shape_type"]).astype(np.float32).reshape(B)
    ident = np.eye(P, dtype=np.float16)
    in_maps = []
    for i in range(N_CORES):
        sl = slice(i * RPC, (i + 1) * RPC)
        in_maps.append({
            "x": x[sl].reshape(P, CPB).copy(),
            "st": st[sl].reshape(P, CPB).copy(),
            "cmA": cmA,
            "cmB": cmB,
            "ident": ident,
        })
    return in_maps


def _get_nc(sig2):
    key = tuple(np.round(sig2, 12))
    if key not in _CACHE:
        _CACHE[key] = _build_nc(sig2)
    return _CACHE[key]


def _get_runner(nc):
    """Cached jit-compiled SPMD executor for `nc` (same mechanics as
    concourse.bass2jax.run_bass_via_pjrt, memoized so repeated kernel()
    calls skip jax re-tracing)."""
    if hasattr(nc, "_cached_runner"):
        return nc._cached_runner
    import jax
    from jax.experimental.shard_map import shard_map
    from jax.sharding import Mesh, PartitionSpec

    import concourse.mybir as mybir
    from concourse import bass2jax

    bass2jax.install_neuronx_cc_hook()

    part_name = (nc.partition_id_tensor.name
                 if nc.partition_id_tensor else None)
    in_names, out_names, out_avals = [], [], []
    for alloc in nc.m.functions[0].allocations:
        if not isinstance(alloc, mybir.MemoryLocationSet):
            continue
        name = alloc.memorylocations[0].name
        if alloc.kind == "ExternalInput":
            if name != part_name:
                in_names.append(name)
        elif alloc.kind == "ExternalOutput":
            out_names.append(name)
            out_avals.append(jax.core.ShapedArray(
                tuple(alloc.tensor_shape), mybir.dt.np(alloc.dtype)))
    n_params = len(in_names)
    all_names = in_names + out_names
    if part_name is not None:
        all_names = all_names + [part_name]
    donate = tuple(range(n_params, n_params + len(out_names)))

    def _body(*args):
        operands = list(args)
        if part_name is not None:
            operands.append(bass2jax.partition_id_tensor())
        return tuple(bass2jax._bass_exec_p.bind(
            *operands,
            out_avals=tuple(out_avals),
            in_names=tuple(all_names),
            out_names=tuple(out_names),
            lowering_input_output_aliases=(),
            sim_require_finite=True,
            sim_require_nnan=True,
            nc=nc,
        ))

    devices = jax.devices()[:N_CORES]
    mesh = Mesh(np.asarray(devices), ("core",))
    sharded = jax.jit(
        shard_map(_body, mesh=mesh,
                  in_specs=(PartitionSpec("core"),) * (n_params + len(out_names)),
                  out_specs=(PartitionSpec("core"),) * len(out_names),
                  check_rep=False),
        donate_argnums=donate, keep_unused=True)
    runner = (sharded, in_names, out_names, out_avals)
    nc._cached_runner = runner
    return runner


def _run_spmd(nc, in_maps):
    sharded, in_names, out_names, out_avals = _get_runner(nc)
    concat_in = [
        np.concatenate([np.asarray(m[name])[None] for m in in_maps], axis=0)
        .reshape(N_CORES * in_maps[0][name].shape[0],
                 *in_maps[0][name].shape[1:])
        for name in in_names
    ]
    concat_zeros = [
        np.zeros((N_CORES * a.shape[0], *a.shape[1:]), a.dtype)
        for a in out_avals
    ]
    out_arrs = sharded(*concat_in, *concat_zeros)
    return {
        name: np.asarray(out_arrs[i]).reshape(
            N_CORES, *out_avals[i].shape)
        for i, name in enumerate(out_names)
    }


def kernel(**inputs) -> np.ndarray:
    if not _assumptions_hold(inputs):
        return _fallback_numpy(inputs)

    cmA, cmB, sig2 = _towers_collapse(inputs)
    nc = _get_nc(sig2)
    in_maps = _make_in_maps(inputs, cmA, cmB)
    y = _run_spmd(nc, in_maps)["y"]            # [N_CORES, P, CPB, TD] f16
    return y.reshape(B, TD).astype(np.float32)



# revision 36
# speedup vs baseline: 1.3396x; 1.0506x over previous
"""Trainium2 Bass kernel for nn_FCLModule_74131135529089 (moe_routing).

Module structure (B=262144 rows, input dim 1):
    circle/rect expert towers 1->32->64->256 (relu, zero biases)
    per-row select by shape_type, stage2 256->256 relu + residual,
    stage3 256->512 relu, 512->512, LayerNorm(512).

All bias vectors in this module are zero and every stage before the
LayerNorm is therefore positively homogeneous in x: for each row,
    h2(x) = |x| * H[k],   k = 2*shape_type + (x < 0),
where H[k] in R^512 is the full pre-LayerNorm output of the network
evaluated at x = +-1 for each expert.  The LayerNorm then collapses to
    out = C[k] * t + ln_b,  C[k] = (H[k]-mean(H[k]))*ln_g,
    t = |x| / sqrt(x^2 * var(H[k]) + eps).
The device kernel computes t/masks per row, forms a [rows,K] one-hot*t
matrix per 128-row chunk (fp16 hi/lo split rows, K padded to 32) and
multiplies it with the constant [K,512] matrix (C rows and ln_b) on the
tensor engine, streaming the output as fp16 (256 MB) at the HBM write
roofline; the host upcasts to f32 during the gather.  If any
structural assumption is violated (nonzero biases / shape_type outside
{0,1}) we fall back to a dense numpy evaluation of the module.

Sharding: pure data parallel over the batch dim, 8 cores x 32768 rows.
"""

import numpy as np

B = 262144
TD = 512
N_CORES = 8
RPC = B // N_CORES          # rows per core = 32768
P = 128                     # SBUF partitions
CPB = RPC // P              # columns per partition = 256 (row r = p*CPB + j)
G = 8                       # 128-row chunks per output DMA (1 MB per DMA)
EPS = 1e-5
# Matmul operands are fp16 with an error-compensating split:
#   t*C = t_hi@C_hi + t_hi@C_lo + t_lo@C_hi   (+ ln_b hi/lo)
# folded into one K=14 matmul; fp16 streams at full PE rate (fp32 is 4x
# slower, float32r is ~11-bit) and the split recovers ~22 mantissa bits.
# K is padded to 16 so EIGHT 128-row chunks batch into one [128,128]
# PE transpose (half the transposes and PSUM->SBUF lhs copies of a
# K=32 layout).  Matmul start partitions must be 32-aligned, so chunk
# PAIRS run as K=32 row-tiled matmuls against two zero-banded constant
# matrices (cmA = [cm;0] bands hits the even chunk, cmB = [0;cm] the
# odd one); the extra MACs land on zeros and PE has headroom.
KDIM = 16
TB = 8                      # chunks per transpose batch
# PSUM->SBUF drain split: ACT_NUM of every SPLIT_DEN chunk-pair copies
# go to the scalar engine, the rest to the vector engine (both drain
# ~1.1us per [P,2,TD] pair; DVE also carries the lhs copies).
ACT_NUM = 70
SPLIT_DEN = 128

_CACHE: dict = {}


def _towers_collapse(inputs):
    """Host-side constant folding (float64): returns the replicated fp16
    constant matrix [128,TD] and sig2 [4] f64, for k = 2*shape_type + (x<0)
    in order (c,+),(c,-),(r,+),(r,-)."""
    W = {k: np.asarray(v, dtype=np.float64) for k, v in inputs.items()}
    H = []
    for e in ("c", "r"):
        for sign in (1.0, -1.0):
            v = np.array([[sign]])
            for li in ("1", "2", "3"):
                v = np.maximum(v @ W[e + "w" + li] + W[e + "b" + li], 0.0)
            x2 = np.maximum(v @ W["s2w"] + W["s2b"], 0.0) + v
            h = np.maximum(x2 @ W["w3a"] + W["b3a"], 0.0)
            H.append((h @ W["w3b"] + W["b3b"])[0])
    # reorder to k = 2*s + neg: (c,+),(c,-),(r,+),(r,-) == H[0],H[1],H[2],H[3]
    H = np.stack(H)                                   # [4, TD]
    mu = H.mean(axis=1, keepdims=True)
    sig2 = H.var(axis=1)                              # [4]
    C = (H - mu) * W["ln_g"][None, :]                 # [4, TD]
    lnb = W["ln_b"]
    # Difference basis: value(n,s) = D0 + n*D1 + s*D2 + n*s*D3 with
    # n = (x<0), s = shape_type.  The device lhs planes are then just
    # {t, t*n, t*s, t*n*s} -- pure multiplies/copies, which is the full
    # extent of what the GpSimd engine supports (no add/sub on Pool).
    D = np.zeros_like(C)
    D[0] = C[0]
    D[1] = C[1] - C[0]
    D[2] = C[2] - C[0]
    D[3] = C[3] - C[1] - C[2] + C[0]
    D_hi = D.astype(np.float16)
    D_lo = (D - D_hi.astype(np.float64)).astype(np.float16)
    b_hi = lnb.astype(np.float16)
    b_lo = (lnb - b_hi.astype(np.float64)).astype(np.float16)
    # rows: 0-3 D_hi (x t_hi planes), 4-7 D_lo (x t_hi planes),
    #       8-11 D_hi (x t_lo planes), 12 b_hi, 13 b_lo, 14-15 zero pad
    cmat = np.zeros((KDIM, TD), np.float16)
    cmat[0:4] = D_hi
    cmat[4:8] = D_lo
    cmat[8:12] = D_hi
    cmat[12] = b_hi
    cmat[13] = b_lo
    # Zero-banded variants for 32-aligned chunk-pair matmuls: band i of
    # cmA holds [cmat; 0] (even chunk of the pair), cmB holds [0; cmat].
    z = np.zeros((KDIM, TD), np.float16)
    cmA = np.tile(np.vstack([cmat, z]), (4, 1))       # [128, TD] f16
    cmB = np.tile(np.vstack([z, cmat]), (4, 1))       # [128, TD] f16
    return np.ascontiguousarray(cmA), np.ascontiguousarray(cmB), sig2


def _assumptions_hold(inputs):
    for name in ("cb1", "cb2", "cb3", "rb1", "rb2", "rb3", "s2b", "b3a", "b3b"):
        if np.any(np.asarray(inputs[name]) != 0):
            return False
    st = np.asarray(inputs["shape_type"])
    if not np.isin(st, (0, 1)).all():
        return False
    x = np.asarray(inputs["x"])
    return bool(np.isfinite(x).all()) and x.shape == (B, 1) and st.shape == (B, 1)


def _fallback_numpy(inputs):
    f = {k: np.asarray(v, dtype=np.float32) for k, v in inputs.items()}

    def tower(h, w1, b1, w2, b2, w3, b3):
        h = np.maximum(h @ w1 + b1, 0)
        h = np.maximum(h @ w2 + b2, 0)
        return np.maximum(h @ w3 + b3, 0)

    x = f["x"]
    circle = tower(x, f["cw1"], f["cb1"], f["cw2"], f["cb2"], f["cw3"], f["cb3"])
    rect = tower(x, f["rw1"], f["rb1"], f["rw2"], f["rb2"], f["rw3"], f["rb3"])
    mask = np.asarray(inputs["shape_type"]) < 0.5
    x1 = np.where(mask, circle, rect)
    x2 = np.maximum(x1 @ f["s2w"] + f["s2b"], 0) + x1
    h = np.maximum(x2 @ f["w3a"] + f["b3a"], 0)
    h = h @ f["w3b"] + f["b3b"]
    mu = h.mean(axis=-1, keepdims=True)
    var = h.var(axis=-1, keepdims=True)
    return ((h - mu) / np.sqrt(var + EPS) * f["ln_g"] + f["ln_b"]).astype(np.float32)


def _build_nc(sig2, reps=1):
    import concourse.bacc as bacc
    import concourse.bass as bass
    import concourse.mybir as mybir
    import concourse.tile as tile

    f32 = mybir.dt.float32
    f16 = mybir.dt.float16
    a = float(sig2[0])
    b = float(sig2[1] - sig2[0])
    c = float(sig2[2] - sig2[0])
    d = float(sig2[3] - sig2[2] - sig2[1] + sig2[0])
    mul = mybir.AluOpType.mult
    add = mybir.AluOpType.add
    sub = mybir.AluOpType.subtract

    nc = bacc.Bacc("TRN2", target_bir_lowering=False, debug=False,
                   num_devices=N_CORES)
    x_d = nc.dram_tensor("x", [P, CPB], f32, kind="ExternalInput").ap()
    s_d = nc.dram_tensor("st", [P, CPB], f32, kind="ExternalInput").ap()
    a_d = nc.dram_tensor("cmA", [P, TD], f16, kind="ExternalInput").ap()
    b_d = nc.dram_tensor("cmB", [P, TD], f16, kind="ExternalInput").ap()
    i_d = nc.dram_tensor("ident", [P, P], f16, kind="ExternalInput").ap()
    # fp16 output stream: halves HBM write traffic (the roofline); host
    # upcasts to f32 after gather.  Final-store rounding is ~2^-11
    # relative, far inside the accuracy gate.
    y_d = nc.dram_tensor("y", [P, CPB, TD], f16, kind="ExternalOutput").ap()

    with tile.TileContext(nc) as tc:
        with (
            tc.tile_pool(name="const", bufs=1) as const,
            tc.tile_pool(name="pre", bufs=1) as pre,
            tc.tile_pool(name="lhs", bufs=6) as lhsp,
            tc.tile_pool(name="outs", bufs=4) as outp,
            tc.tile_pool(name="ps_t", bufs=2, space="PSUM") as ps_t,
            tc.tile_pool(name="ps_o", bufs=3, space="PSUM") as ps_o,
        ):
            xr = pre.tile([P, CPB], f32)
            sf = pre.tile([P, CPB], f32)
            neg = pre.tile([P, CPB], f32)
            sn = pre.tile([P, CPB], f32)
            u1 = pre.tile([P, CPB], f32)
            u2 = pre.tile([P, CPB], f32)
            sg = pre.tile([P, CPB], f32)
            x2 = pre.tile([P, CPB], f32)
            ve = pre.tile([P, CPB], f32)
            ve2 = pre.tile([P, CPB], f32)
            rc = pre.tile([P, CPB], f32)
            t2 = pre.tile([P, CPB], f32)
            tt = pre.tile([P, CPB], f32)
            m4 = pre.tile([P, CPB, KDIM], f16)

            # The PSUM f32 -> SBUF f16 drain (16.7M elem/core) is the
            # engine-side critical path under the fp16 DMA stream: both
            # ACT and DVE drain ~1 elem/cycle/lane, so chunk-pair copies
            # are interleaved ACT_NUM:SPLIT_DEN-ACT_NUM between them
            # (DVE also carries the lhs copies; everything else is off
            # these two engines).
            # [total chunks drained, chunks drained by ACT]; seeded
            # so the first drains land on ACT, which is idle during the
            # ramp while DVE clears prep work
            drain_state = [0, -4]

            def drain(dst, pp, nch):
                # chunk-weighted split between ACT and DVE: send this
                # set to whichever engine is furthest below its target
                # share of drained chunks
                total, act = drain_state
                if (act + nch) * SPLIT_DEN <= (total + nch) * ACT_NUM:
                    drain_state[1] = act + nch
                    nc.scalar.copy(dst, pp)
                else:
                    nc.vector.tensor_copy(dst, pp)
                drain_state[0] = total + nch

            def mm(pp_col, lh, pos, cmx):
                band = 32 * (pos // 2)
                nc.tensor.matmul(
                    pp_col, lh[band:band + 32, :], cmx[band:band + 32, :],
                    start=True, stop=True, tile_position=(band, 0))

            def emit_group(g0, gsz):
                outt = outp.tile([P, G, TD], f16, tag="outt")
                tp = ps_t.tile([P, P], f16, tag="tp")
                nc.tensor.transpose(
                    tp[:KDIM * gsz], m4[:, g0:g0 + gsz, :], idt[:])
                lh = lhsp.tile([P, P], f16, tag="lh")
                nc.vector.tensor_copy(lh[:KDIM * gsz], tp[:KDIM * gsz])
                if gsz == G:
                    # Cross-band pairing (0,2),(4,6),(1,3),(5,7): each
                    # PSUM pair spans two different PE row bands and the
                    # matmul sequence rotates 0,32,64,96,... so
                    # consecutive matmuls stream concurrently instead of
                    # serializing on one band.  cmA/cmB zero banding
                    # selects the chunk's 16 planes within its band.
                    pairs = ((0, 2), (4, 6), (1, 3), (5, 7))
                else:
                    pairs = tuple((a, a + 1) for a in range(0, gsz, 2))
                for ca, cb in pairs:
                    pp = ps_o.tile([P, 2, TD], f32, tag="po")
                    mm(pp[:, 0, :], lh, ca, cmA if ca % 2 == 0 else cmB)
                    mm(pp[:, 1, :], lh, cb, cmA if cb % 2 == 0 else cmB)
                    dl = cb - ca
                    drain(outt[:, ca:cb + 1:dl, :], pp[:], 2)
                # alternate output DMAs between the Sync HWDGE queue and
                # the (otherwise idle) GpSimd SWDGE queue so per-DMA
                # issue latency pipelines across two issuing engines
                if (g0 // G) % 2 == 0:
                    nc.sync.dma_start(y_d[:, g0:g0 + gsz, :],
                                      outt[:, 0:gsz, :])
                else:
                    nc.gpsimd.dma_start(y_d[:, g0:g0 + gsz, :],
                                        outt[:, 0:gsz, :])

            nc.sync.dma_start(xr[:], x_d[:])
            nc.scalar.dma_start(sf[:], s_d[:])
            cmA = const.tile([P, TD], f16)
            nc.scalar.dma_start(cmA[:], a_d[:])
            cmB = const.tile([P, TD], f16)
            nc.scalar.dma_start(cmB[:], b_d[:])
            idt = const.tile([P, P], f16)
            nc.sync.dma_start(idt[:], i_d[:])

            # constant planes, valid for every column: 8-11 zero
            # (dropped t_lo planes), 12/13 all-ones (bias rows), 14/15
            # zero pad; cheap one-time Pool memsets
            nc.gpsimd.memset(m4[:, :, 8:12], 0.0)
            nc.gpsimd.memset(m4[:, :, 12:14], 1.0)
            nc.gpsimd.memset(m4[:, :, 14:KDIM], 0.0)

            # Per-row preprocessing.  The Pool (GpSimd) engine only
            # supports mult/copy, so it carries exactly those (the m4
            # plane products are the bulk); DVE keeps the few
            # add/sub/compare ops, ACT the +eps and Sqrt.  Slice 0 runs
            # fully on DVE for a fast ramp.  Each later slice is split
            # into three emission phases (pre / mid / post) placed
            # between groups so the short DVE/ACT mid-chain ops sit in
            # their FIFOs close to where their cross-engine inputs
            # complete, without blocking queued drains for long.
            bounds = [0, 32, 96, 176, CPB]
            nslices = len(bounds) - 1

            def prep_pre(h):
                e = nc.vector if h == 0 else nc.gpsimd
                cs = slice(bounds[h], bounds[h + 1])
                nc.vector.tensor_scalar(neg[:, cs], xr[:, cs], 0.0, None,
                                        mybir.AluOpType.is_lt)
                # sig2 per row: a + b*neg + c*sf + d*sn
                nc.vector.tensor_scalar(u1[:, cs], sf[:, cs], c, a, mul, add)
                nc.vector.scalar_tensor_tensor(u2[:, cs], neg[:, cs], b,
                                               u1[:, cs], mul, add)
                e.tensor_tensor(sn[:, cs], sf[:, cs], neg[:, cs], mul)
                nc.vector.scalar_tensor_tensor(sg[:, cs], sn[:, cs], d,
                                               u2[:, cs], mul, add)
                e.tensor_tensor(x2[:, cs], xr[:, cs], xr[:, cs], mul)
                e.tensor_tensor(ve[:, cs], x2[:, cs], sg[:, cs], mul)

            def prep_mid(h):
                e = nc.vector if h == 0 else nc.gpsimd
                cs = slice(bounds[h], bounds[h + 1])
                # t = sqrt(x^2 / (x^2*sig2 + eps))
                nc.scalar.activation(ve2[:, cs], ve[:, cs],
                                     mybir.ActivationFunctionType.Copy,
                                     bias=EPS, scale=1.0)
                nc.vector.reciprocal(rc[:, cs], ve2[:, cs])
                e.tensor_tensor(t2[:, cs], x2[:, cs], rc[:, cs], mul)
                nc.scalar.activation(tt[:, cs], t2[:, cs],
                                     mybir.ActivationFunctionType.Sqrt)

            def prep_post(h):
                e = nc.vector if h == 0 else nc.gpsimd
                cs = slice(bounds[h], bounds[h + 1])
                # planes (difference basis): {t, t*n, t*s, t*n*s} for
                # D_hi, duplicated for D_lo.  (No t_lo compensation:
                # t_hi rounding costs <= 2^-11 relative, inside the
                # accuracy budget, and dropping it removes the whole
                # th16/th32/tl cross-engine chain.)
                e.tensor_copy(m4[:, cs, 0], tt[:, cs])
                e.tensor_tensor(m4[:, cs, 1], tt[:, cs], neg[:, cs], mul)
                e.tensor_tensor(m4[:, cs, 2], tt[:, cs], sf[:, cs], mul)
                e.tensor_tensor(m4[:, cs, 3], tt[:, cs], sn[:, cs], mul)
                e.tensor_copy(m4[:, cs, 4:8], m4[:, cs, 0:4])

            def slice_groups(h):
                # ramped small at the start (first DMA out early),
                # small again at the very end (short drain+DMA tail)
                if h == 0:
                    groups = [(0, 2), (2, 2), (4, 4)]
                    g = 8
                else:
                    groups = []
                    g = bounds[h]
                end = bounds[h + 1] if h + 1 < nslices else CPB - 8
                while g < end:
                    groups.append((g, G))
                    g += G
                if h == nslices - 1:
                    groups += [(g, 4), (g + 4, 2), (g + 6, 2)]
                return groups

            prep_pre(0)
            prep_mid(0)
            prep_post(0)
            for h in range(nslices):
                gs = slice_groups(h)
                for i, (g0, gsz) in enumerate(gs):
                    emit_group(g0, gsz)
                    if h + 1 < nslices:
                        # stage the next slice's prep across this
                        # slice's first few groups; for late slices,
                        # wait-gate the scheduler (simulated-time lower
                        # bound, no hardware cost) so their mid-chain
                        # ACT/DVE ops can't be hoisted ahead of early
                        # drains and head-of-line block those engines
                        gate = max(0.0, bounds[h + 1] * 0.35e-3
                                   - 18e-3)
                        if i == 0:
                            prep_pre(h + 1)
                        elif i == 1:
                            with tc.tile_wait_until(gate, enable=gate > 0):
                                prep_mid(h + 1)
                        elif i == 2:
                            with tc.tile_wait_until(gate, enable=gate > 0):
                                prep_post(h + 1)

            # extra full passes for repeat-based HW timing (reps > 1)
            for _ in range(reps - 1):
                for g0 in range(0, CPB, G):
                    emit_group(g0, G)
    nc.compile()
    return nc


def _make_in_maps(inputs, cmA, cmB):
    x = np.ascontiguousarray(np.asarray(inputs["x"], dtype=np.float32)).reshape(B)
    st = np.asarray(inputs["shape_type"]).astype(np.float32).reshape(B)
    ident = np.eye(P, dtype=np.float16)
    in_maps = []
    for i in range(N_CORES):
        sl = slice(i * RPC, (i + 1) * RPC)
        in_maps.append({
            "x": x[sl].reshape(P, CPB).copy(),
            "st": st[sl].reshape(P, CPB).copy(),
            "cmA": cmA,
            "cmB": cmB,
            "ident": ident,
        })
    return in_maps


def _get_nc(sig2):
    key = tuple(np.round(sig2, 12))
    if key not in _CACHE:
        _CACHE[key] = _build_nc(sig2)
    return _CACHE[key]


def _get_runner(nc):
    """Cached jit-compiled SPMD executor for `nc` (same mechanics as
    concourse.bass2jax.run_bass_via_pjrt, memoized so repeated kernel()
    calls skip jax re-tracing)."""
    if hasattr(nc, "_cached_runner"):
        return nc._cached_runner
    import jax
    from jax.experimental.shard_map import shard_map
    from jax.sharding import Mesh, PartitionSpec

    import concourse.mybir as mybir
    from concourse import bass2jax

    bass2jax.install_neuronx_cc_hook()

    part_name = (nc.partition_id_tensor.name
                 if nc.partition_id_tensor else None)
    in_names, out_names, out_avals = [], [], []
    for alloc in nc.m.functions[0].allocations:
        if not isinstance(alloc, mybir.MemoryLocationSet):
            continue
        name = alloc.memorylocations[0].name
        if alloc.kind == "ExternalInput":
            if name != part_name:
                in_names.append(name)
        elif alloc.kind == "ExternalOutput":
            out_names.append(name)
            out_avals.append(jax.core.ShapedArray(
                tuple(alloc.tensor_shape), mybir.dt.np(alloc.dtype)))
    n_params = len(in_names)
    all_names = in_names + out_names
    if part_name is not None:
        all_names = all_names + [part_name]
    donate = tuple(range(n_params, n_params + len(out_names)))

    def _body(*args):
        operands = list(args)
        if part_name is not None:
            operands.append(bass2jax.partition_id_tensor())
        return tuple(bass2jax._bass_exec_p.bind(
            *operands,
            out_avals=tuple(out_avals),
            in_names=tuple(all_names),
            out_names=tuple(out_names),
            lowering_input_output_aliases=(),
            sim_require_finite=True,
            sim_require_nnan=True,
            nc=nc,
        ))

    devices = jax.devices()[:N_CORES]
    mesh = Mesh(np.asarray(devices), ("core",))
    sharded = jax.jit(
        shard_map(_body, mesh=mesh,
                  in_specs=(PartitionSpec("core"),) * (n_params + len(out_names)),
                  out_specs=(PartitionSpec("core"),) * len(out_names),
                  check_rep=False),
        donate_argnums=donate, keep_unused=True)
    runner = (sharded, in_names, out_names, out_avals)
    nc._cached_runner = runner
    return runner


def _run_spmd(nc, in_maps):
    sharded, in_names, out_names, out_avals = _get_runner(nc)
    concat_in = [
        np.concatenate([np.asarray(m[name])[None] for m in in_maps], axis=0)
        .reshape(N_CORES * in_maps[0][name].shape[0],
                 *in_maps[0][name].shape[1:])
        for name in in_names
    ]
    concat_zeros = [
        np.zeros((N_CORES * a.shape[0], *a.shape[1:]), a.dtype)
        for a in out_avals
    ]
    out_arrs = sharded(*concat_in, *concat_zeros)
    return {
        name: np.asarray(out_arrs[i]).reshape(
            N_CORES, *out_avals[i].shape)
        for i, name in enumerate(out_names)
    }


def kernel(**inputs) -> np.ndarray:
    if not _assumptions_hold(inputs):
        return _fallback_numpy(inputs)

    cmA, cmB, sig2 = _towers_collapse(inputs)
    nc = _get_nc(sig2)
    in_maps = _make_in_maps(inputs, cmA, cmB)
    y = _run_spmd(nc, in_maps)["y"]            # [N_CORES, P, CPB, TD] f16
    return y.reshape(B, TD).astype(np.float32)

